# revision 5
# baseline (speedup 1.0000x reference)
"""Haar DWT pooling (NHWC, 2x2 blocks, all 4 components channel-interleaved).

Full input x: (8, 512, 512, 64) f32 -> output (8, 256, 256, 256) f32.
Sharding: data-parallel over batch; core b handles x[b] (no communication).

Per-core dataflow (x_b: (512,512,64) -> y_b: (256,256,256)):
  - partition p <-> input columns [4p, 4p+4)  (= output pixels 2p, 2p+1)
  - loop over chunks of K input rows:
      load   X[128, K*256]   <- x[h0:h0+K]           (1KB runs per partition)
      DVE    s = r0+r1 (to S tile), d = r0-r1 (in place over r1)
             -- vertical butterfly, 2 ops over all columns at once
      DVE    ll=s0+s1 lh=d0+d1 hl=s0-s1 hh=d0-d1     (horizontal butterfly,
                                                      written at stride 4 into
                                                      the c*4+comp layout)
      ACT    OUT *= 0.5                              (in-place, idle engine)
      store  OUT[128, K*256] -> y[i0:i0+K/2]         (2KB runs per partition)

fp32 tensor_tensor on DVE is hard-capped at 1 elem/lane/cycle, so the kernel
minimizes DVE instruction count (6 tensor ops per chunk) and per-op overhead.
"""

import numpy as np

import concourse.bacc as bacc
import concourse.mybir as mybir
from concourse.bass_utils import run_bass_kernel_spmd
from concourse.tile import TileContext

N_CORES = 8
H = 512
W = 512
C = 64
P = 128  # SBUF partitions; each covers W/P = 4 input columns


def build_dwt_body(nc, tc, x_ap, out_ap, rows_per_chunk=16, io_bufs=3, mid_bufs=2):
    """Emit the per-core DWT pooling kernel body under an open TileContext.

    x_ap:   DRAM AP, shape (H, W, C) f32 (H divisible by rows_per_chunk)
    out_ap: DRAM AP, shape (H//2, W//2, 4*C) f32
    """
    K = rows_per_chunk
    h_total = x_ap.shape[0]
    assert x_ap.shape == (h_total, W, C)
    assert out_ap.shape == (h_total // 2, W // 2, 4 * C)
    assert h_total % K == 0 and K % 2 == 0
    n_chunks = h_total // K
    M = K // 2  # output rows per chunk

    dt = mybir.dt.float32
    with (
        tc.tile_pool(name="io", bufs=io_bufs) as io_pool,
        tc.tile_pool(name="mid", bufs=mid_bufs) as mid_pool,
    ):
        for ci in range(n_chunks):
            h0 = ci * K
            i0 = ci * M

            # ---- load: x[h0:h0+K] -> X[p, k, wc] (per-partition 1KB runs)
            xt = io_pool.tile([P, K * 256], dt)
            nc.sync.dma_start(
                out=xt[:].rearrange("p (k wc) -> p k wc", wc=256),
                in_=x_ap[h0 : h0 + K].rearrange("k (p w) c -> p k (w c)", p=P),
            )

            # X free-dim layout per partition: (m, k2, wc) with wc = (jl, wp, c)
            #   k = 2m+k2 (row pair m, row-in-pair k2), w4 = 2*jl+wp
            xr = xt[:].rearrange("p (m k2 wc) -> p m k2 wc", k2=2, wc=256)
            r0 = xr[:, :, 0, :]  # rows 2i   : (a | b) interleaved over wp
            r1 = xr[:, :, 1, :]  # rows 2i+1 : (c | d)

            # ---- stage 1: vertical butterfly over all columns at once
            #   s = r0 + r1 -> S tile;  d = r0 - r1 -> in place over r1
            st = mid_pool.tile([P, M * 256], dt)
            sv = st[:].rearrange("p (m wc) -> p m wc", wc=256)
            nc.vector.tensor_add(sv, r0, r1)
            nc.vector.tensor_sub(r1, r0, r1)

            # views splitting even/odd columns: (m, jl, c)
            s_ = st[:].rearrange("p (m jl wp c) -> p m jl wp c", jl=2, wp=2, c=C)
            d_ = xt[:].rearrange("p (m k2 jl wp c) -> p m k2 jl wp c", k2=2, jl=2, wp=2, c=C)
            s0 = s_[:, :, :, 0, :]
            s1 = s_[:, :, :, 1, :]
            d0 = d_[:, :, 1, :, 0, :]
            d1 = d_[:, :, 1, :, 1, :]

            # ---- stage 2: horizontal butterfly into comp-planar scratch.
            #      DVE writes at stride 4 run at ~half rate, so keep DVE
            #      writes dense and let ACT do the interleave below.
            o2 = mid_pool.tile([P, 4 * M * 128], dt)
            o2v = o2[:].rearrange("p (comp m jl c) -> p comp m jl c", comp=4, jl=2, c=C)
            nc.vector.tensor_add(o2v[:, 0], s0, s1)  # LL = s0+s1
            nc.vector.tensor_add(o2v[:, 1], d0, d1)  # LH = d0+d1
            nc.vector.tensor_sub(o2v[:, 2], s0, s1)  # HL = s0-s1
            nc.vector.tensor_sub(o2v[:, 3], d0, d1)  # HH = d0-d1

            # ---- scale by 0.5 + channel interleave on the idle ACT engine:
            #      comp plane (dense read) -> (c*4 + comp) slots (strided write)
            ot = io_pool.tile([P, M * 512], dt)
            ov = ot[:].rearrange("p (m jl c comp) -> p m jl c comp", jl=2, c=C, comp=4)
            for comp in range(4):
                nc.scalar.mul(ov[:, :, :, :, comp], o2v[:, comp], 0.5)

            # ---- store: OUT[p, i, jc] -> out[i0:i0+M] (per-partition 2KB runs)
            nc.sync.dma_start(
                out=out_ap[i0 : i0 + M].rearrange("i (p j) c -> p i (j c)", p=P),
                in_=ot[:].rearrange("p (i jc) -> p i jc", jc=512),
            )


def build_bass(h=H, rows_per_chunk=16, io_bufs=3, mid_bufs=2):
    nc = bacc.Bacc(trn_type="TRN2", target_bir_lowering=False, debug=False)
    x_d = nc.dram_tensor("x", [h, W, C], mybir.dt.float32, kind="ExternalInput")
    out_d = nc.dram_tensor(
        "out", [h // 2, W // 2, 4 * C], mybir.dt.float32, kind="ExternalOutput"
    )
    with TileContext(nc) as tc:
        build_dwt_body(
            nc, tc, x_d.ap(), out_d.ap(),
            rows_per_chunk=rows_per_chunk, io_bufs=io_bufs, mid_bufs=mid_bufs,
        )
    nc.finalize()
    return nc


_NC_CACHE = {}


def _get_nc():
    if "nc" not in _NC_CACHE:
        _NC_CACHE["nc"] = build_bass()
    return _NC_CACHE["nc"]


def run_spmd(x, **kwargs):
    """Run the 8-core SPMD kernel on full input x (8,512,512,64).

    Returns (output (8,256,256,256) f32, BassKernelResults)."""
    x = np.asarray(x)
    assert x.shape == (N_CORES, H, W, C) and x.dtype == np.float32
    nc = _get_nc()
    in_maps = [{"x": np.ascontiguousarray(x[b])} for b in range(N_CORES)]
    res = run_bass_kernel_spmd(nc, in_maps, core_ids=list(range(N_CORES)), **kwargs)
    out = np.stack([res.results[b]["out"] for b in range(N_CORES)], axis=0)
    return out, res


def kernel(x):
    out, _ = run_spmd(x)
    return out


# revision 6
# speedup vs baseline: 1.2048x; 1.2048x over previous
"""Haar DWT pooling (NHWC, 2x2 blocks, all 4 components channel-interleaved).

Full input x: (8, 512, 512, 64) f32 -> output (8, 256, 256, 256) f32.
Sharding: data-parallel over batch; core b handles x[b] (no communication).

Per-core dataflow (x_b: (512,512,64) -> y_b: (256,256,256)):
  - partition p <-> input columns [4p, 4p+4)  (= output pixels 2p, 2p+1)
  - loop over chunks of K input rows:
      load   X[128, K*256]   <- x[h0:h0+K]           (1KB runs per partition)
      DVE    s = r0+r1 (to S tile), d = r0-r1 (in place over r1)
             -- vertical butterfly, 2 ops over all columns at once
      DVE    ll=s0+s1 lh=d0+d1 hl=s0-s1 hh=d0-d1     (horizontal butterfly,
                                                      written at stride 4 into
                                                      the c*4+comp layout)
      ACT    OUT *= 0.5                              (in-place, idle engine)
      store  OUT[128, K*256] -> y[i0:i0+K/2]         (2KB runs per partition)

fp32 tensor_tensor on DVE is hard-capped at 1 elem/lane/cycle, so the kernel
minimizes DVE instruction count (6 tensor ops per chunk) and per-op overhead.
"""

import numpy as np

import concourse.bacc as bacc
import concourse.mybir as mybir
from concourse.bass_utils import run_bass_kernel_spmd
from concourse.tile import TileContext

N_CORES = 8
H = 512
W = 512
C = 64
P = 128  # SBUF partitions; each covers W/P = 4 input columns


def build_dwt_body(nc, tc, x_ap, out_ap, rows_per_chunk=16, io_bufs=3, mid_bufs=2):
    """Emit the per-core DWT pooling kernel body under an open TileContext.

    x_ap:   DRAM AP, shape (H, W, C) f32 (H divisible by rows_per_chunk)
    out_ap: DRAM AP, shape (H//2, W//2, 4*C) f32
    """
    K = rows_per_chunk
    h_total = x_ap.shape[0]
    assert x_ap.shape == (h_total, W, C)
    assert out_ap.shape == (h_total // 2, W // 2, 4 * C)
    assert h_total % K == 0 and K % 2 == 0
    n_chunks = h_total // K
    M = K // 2  # output rows per chunk

    dt = mybir.dt.float32
    with (
        tc.tile_pool(name="io", bufs=io_bufs) as io_pool,
        tc.tile_pool(name="mid", bufs=mid_bufs) as mid_pool,
    ):
        for ci in range(n_chunks):
            h0 = ci * K
            i0 = ci * M

            # ---- load: x[h0:h0+K] -> X[p, k, wc] (per-partition 1KB runs)
            xt = io_pool.tile([P, K * 256], dt)
            nc.sync.dma_start(
                out=xt[:].rearrange("p (k wc) -> p k wc", wc=256),
                in_=x_ap[h0 : h0 + K].rearrange("k (p w) c -> p k (w c)", p=P),
            )

            # X free-dim layout per partition: (m, k2, wc) with wc = (jl, wp, c)
            #   k = 2m+k2 (row pair m, row-in-pair k2), w4 = 2*jl+wp
            xr = xt[:].rearrange("p (m k2 wc) -> p m k2 wc", k2=2, wc=256)
            r0 = xr[:, :, 0, :]  # rows 2i   : (a | b) interleaved over wp
            r1 = xr[:, :, 1, :]  # rows 2i+1 : (c | d)

            # ---- stage 1: vertical butterfly over all columns at once
            #   s = r0 + r1 -> S tile;  d = r0 - r1 -> in place over r1
            st = mid_pool.tile([P, M * 256], dt)
            sv = st[:].rearrange("p (m wc) -> p m wc", wc=256)
            nc.vector.tensor_add(sv, r0, r1)
            nc.vector.tensor_sub(r1, r0, r1)

            # views splitting even/odd columns: (m, jl, c)
            s_ = st[:].rearrange("p (m jl wp c) -> p m jl wp c", jl=2, wp=2, c=C)
            d_ = xt[:].rearrange("p (m k2 jl wp c) -> p m k2 jl wp c", k2=2, jl=2, wp=2, c=C)
            s0 = s_[:, :, :, 0, :]
            s1 = s_[:, :, :, 1, :]
            d0 = d_[:, :, 1, :, 0, :]
            d1 = d_[:, :, 1, :, 1, :]

            # ---- stage 2: horizontal butterfly into comp-planar scratch.
            #      DVE writes at stride 4 run at ~half rate, so keep DVE
            #      writes dense and let ACT do the interleave below.
            o2 = mid_pool.tile([P, 4 * M * 128], dt)
            o2v = o2[:].rearrange("p (comp m jl c) -> p comp m jl c", comp=4, jl=2, c=C)
            nc.vector.tensor_add(o2v[:, 0], s0, s1)  # LL = s0+s1
            nc.vector.tensor_add(o2v[:, 1], d0, d1)  # LH = d0+d1
            nc.vector.tensor_sub(o2v[:, 2], s0, s1)  # HL = s0-s1
            nc.vector.tensor_sub(o2v[:, 3], d0, d1)  # HH = d0-d1

            # ---- scale by 0.5 + channel interleave on the idle ACT engine:
            #      comp plane (dense read) -> (c*4 + comp) slots (strided write)
            ot = io_pool.tile([P, M * 512], dt)
            ov = ot[:].rearrange("p (m jl c comp) -> p m jl c comp", jl=2, c=C, comp=4)
            for comp in range(4):
                nc.scalar.mul(ov[:, :, :, :, comp], o2v[:, comp], 0.5)

            # ---- store: OUT[p, i, jc] -> out[i0:i0+M] (per-partition 2KB runs)
            # nc.scalar (ACT) HWDGE ring, separate from the load ring on
            # nc.sync (SP) — sharing one FIFO ring head-of-line-blocks loads
            # behind stores that wait on compute.
            nc.scalar.dma_start(
                out=out_ap[i0 : i0 + M].rearrange("i (p j) c -> p i (j c)", p=P),
                in_=ot[:].rearrange("p (i jc) -> p i jc", jc=512),
            )


def build_bass(h=H, rows_per_chunk=16, io_bufs=3, mid_bufs=2):
    nc = bacc.Bacc(trn_type="TRN2", target_bir_lowering=False, debug=False)
    x_d = nc.dram_tensor("x", [h, W, C], mybir.dt.float32, kind="ExternalInput")
    out_d = nc.dram_tensor(
        "out", [h // 2, W // 2, 4 * C], mybir.dt.float32, kind="ExternalOutput"
    )
    with TileContext(nc) as tc:
        build_dwt_body(
            nc, tc, x_d.ap(), out_d.ap(),
            rows_per_chunk=rows_per_chunk, io_bufs=io_bufs, mid_bufs=mid_bufs,
        )
    nc.finalize()
    return nc


_NC_CACHE = {}


def _get_nc():
    if "nc" not in _NC_CACHE:
        _NC_CACHE["nc"] = build_bass()
    return _NC_CACHE["nc"]


def run_spmd(x, **kwargs):
    """Run the 8-core SPMD kernel on full input x (8,512,512,64).

    Returns (output (8,256,256,256) f32, BassKernelResults)."""
    x = np.asarray(x)
    assert x.shape == (N_CORES, H, W, C) and x.dtype == np.float32
    nc = _get_nc()
    in_maps = [{"x": np.ascontiguousarray(x[b])} for b in range(N_CORES)]
    res = run_bass_kernel_spmd(nc, in_maps, core_ids=list(range(N_CORES)), **kwargs)
    out = np.stack([res.results[b]["out"] for b in range(N_CORES)], axis=0)
    return out, res


def kernel(x):
    out, _ = run_spmd(x)
    return out
